# revision 1
# baseline (speedup 1.0000x reference)
"""Complex multi-head attention on 8 Trainium2 cores (Bass/Tile).

Sharding: pure data-parallel over batch (B=8 -> 1 batch per core),
weights replicated. No collectives.

Per-core dataflow (batch b), all matmuls float32r (full rate at N=512):
  - Host supplies feature-major activations XT = [xr.T; xi.T] [1024, S]
    and repacked/sign-folded weights so every complex linear is one
    stacked-K real matmul chain.
  - V-projection (all heads) -> V1 token-major [t, (h, vr|vi)].
  - Per head h: Q/K projections -> feature-major stacks [(c,dh)=128, S];
    scores computed TRANSPOSED (S.T = K-stationary) so softmax'd scores
    feed the AV matmul directly (no transposes anywhere);
    softmax without max-subtraction (|s| <= ~16, exp safe in fp32);
    row sums via ones-matmuls packed into one PSUM bank (tile_position);
    normalization fused into the P1/P2 PSUM evacuation via
    broadcast-DMA'd reciprocals.
  - Output projection accumulates heads as K-chunks -> [t, (o, c)] which
    is exactly the [S, D, 2] DRAM layout.
"""

import sys
import types
import numpy as np

B, S, D, H = 8, 1024, 512, 8
DH = D // H
KC = 8  # k-chunks of 128 over (c,d) = 1024
TC = 8  # token chunks of 128
NCORES = 8

LAST_EXEC_NS = None


# ---------------------------------------------------------------- shims
def _install_axon_profile_shim():
    if "antenv.axon_hooks" in sys.modules:
        return
    try:
        import antenv  # noqa: F401

        mod = types.ModuleType("antenv.axon_hooks")
        state = {"hook": None}
        mod.set_axon_ntff_profile_hook = lambda h: state.__setitem__("hook", h)
        mod.get_axon_ntff_profile_hook = lambda: state["hook"]
        sys.modules["antenv.axon_hooks"] = mod
        from trn_agent_boot.trn_boot import _ntff_profile_via_ctypes

        hook = _ntff_profile_via_ctypes("/opt/axon/libaxon_pjrt.so")
        if hook is not None:
            mod.set_axon_ntff_profile_hook(hook)
    except Exception:
        pass


def _install_tile_drain_patch():
    """This walrus build allows ONE sync wait per instruction; split the
    TileContext exit drain's waits across preceding sync NOPs."""
    import concourse.mybir as mybir
    import concourse.tile as tile
    from concourse.vector_clock import ScopedClock

    if getattr(tile.TileContext, "_drain_patched", False):
        return

    def _patched(self, tick_clock, wait_clock):
        probe = mybir.InstNoOp(name="I-drain-probe")
        probe.engine = mybir.EngineType.SP
        wait_clock.add_sem_waits(probe, ScopedClock({None: tick_clock.global_clock}))
        waits = list(probe.sync_info.on_wait or []) if probe.sync_info else []
        for w in waits:
            nop = self.nc.sync.nop()
            nop.ins.sync_info = mybir.SyncInfo(on_wait=[w], on_update=[])
        self.nc.sync.drain()
        self.nc.all_engine_barrier()
        assert self.sems is not None
        popped = self.nc._tile_sem_poison_stack.pop()
        assert popped is self._sem_poison
        self.nc.clear_and_free_semaphores(list(self.sems.allocated().values()))
        self.nc.all_engine_barrier()

    tile.TileContext._drain_and_barrier = _patched
    tile.TileContext._drain_patched = True


def _split_waits(nc, max_waits=1):
    """Hoist extra sync waits onto preceding same-engine NOPs (walrus here
    rejects >1 sync wait per instruction)."""
    import concourse.mybir as mybir

    def process(blk):
        lst = blk.instructions
        i = 0
        while i < len(lst):
            inst = lst[i]
            if hasattr(inst, "blocks"):
                for b in inst.blocks or []:
                    process(b)
            si = inst.sync_info
            if si is not None and si.on_wait and len(si.on_wait) > max_waits:
                waits = list(si.on_wait)
                keep, extra = waits[-max_waits:], waits[:-max_waits]
                inst.sync_info = mybir.SyncInfo(
                    on_wait=keep, on_update=list(si.on_update or [])
                )
                for j, w in enumerate(extra):
                    nop = mybir.InstNoOp(name=f"{inst.name}-ws{j}")
                    nop.engine = inst.engine
                    nop.sync_info = mybir.SyncInfo(on_wait=[w], on_update=[])
                    lst.insert(i, nop)
                    i += 1
            i += 1

    for f in nc.m.functions:
        for blk in f.blocks:
            process(blk)


# ------------------------------------------------------------ host prep
def _build_wqk(wr, wi, scale):
    """[1024 k=(c,d), 1024 m=(h, c', dh)] for Q/K projections."""
    W = np.empty((2 * D, 2 * D), np.float32)
    for h in range(H):
        o = slice(h * DH, (h + 1) * DH)
        c0 = h * 2 * DH
        W[0:D, c0 : c0 + DH] = wr[o].T * scale
        W[D:, c0 : c0 + DH] = -wi[o].T * scale
        W[0:D, c0 + DH : c0 + 2 * DH] = wi[o].T * scale
        W[D:, c0 + DH : c0 + 2 * DH] = wr[o].T * scale
    return W


def _head_tiles(W):
    """[1024,1024] -> [H, 128, 1024]: per-head column block, k-chunk cols."""
    out = np.empty((H, 128, 1024), np.float32)
    for h in range(H):
        blk = W[:, h * 128 : (h + 1) * 128]  # [1024, 128]
        for kk in range(KC):
            out[h, :, kk * 128 : (kk + 1) * 128] = blk[kk * 128 : (kk + 1) * 128]
    return out


def _kchunk_tiles(W):
    """[1024,1024] -> [KC, 128, 1024]: row chunks."""
    return np.ascontiguousarray(W.reshape(KC, 128, 1024))


def _build_wo(wo_r, wo_i):
    """rows (h, c', dh), cols (o, c) interleaved to match [S, D, 2]."""
    W = np.empty((2 * D, 2 * D), np.float32)
    for h in range(H):
        d = slice(h * DH, (h + 1) * DH)
        r0 = h * 2 * DH
        W[r0 : r0 + DH, 0::2] = wo_r[:, d].T
        W[r0 : r0 + DH, 1::2] = wo_i[:, d].T
        W[r0 + DH : r0 + 2 * DH, 0::2] = -wo_i[:, d].T
        W[r0 + DH : r0 + 2 * DH, 1::2] = wo_r[:, d].T
    return W


def _xt(x):  # [S, D, 2] -> [2D, S] feature-major
    out = np.empty((2 * D, S), np.float32)
    out[0:D] = x[:, :, 0].T
    out[D:] = x[:, :, 1].T
    return out


# ------------------------------------------------------------ bass build
def _build_nc():
    import concourse.bass as bass
    import concourse.bass as bass_mod
    import concourse.mybir as mybir
    import concourse.tile as tile
    from contextlib import ExitStack

    MDT = mybir.dt.float32r
    F32 = mybir.dt.float32

    nc = bass.Bass()
    d_xtq = nc.dram_tensor("xtq", [KC, 128, S], MDT, kind="ExternalInput")
    d_xtk = nc.dram_tensor("xtk", [KC, 128, S], MDT, kind="ExternalInput")
    d_xtv = nc.dram_tensor("xtv", [KC, 128, S], MDT, kind="ExternalInput")
    d_wq = nc.dram_tensor("wq", [H, 128, 1024], MDT, kind="ExternalInput")
    d_wk = nc.dram_tensor("wk", [H, 128, 1024], MDT, kind="ExternalInput")
    d_wv = nc.dram_tensor("wv", [KC, 128, 1024], MDT, kind="ExternalInput")
    d_wo = nc.dram_tensor("wo", [H, 128, 1024], MDT, kind="ExternalInput")
    d_cst = nc.dram_tensor("cst", [128, 320], MDT, kind="ExternalInput")
    d_out = nc.dram_tensor("out", [S, 1024], F32, kind="ExternalOutput")

    with tile.TileContext(nc) as tc, ExitStack() as ctx:
        ctx.enter_context(
            nc.allow_low_precision(reason="float32r tiles are bit-identical fp32")
        )
        pA = ctx.enter_context(tc.tile_pool(name="bigA", bufs=8))  # xtv -> xtq -> wo
        pB = ctx.enter_context(tc.tile_pool(name="bigB", bufs=8))  # wv -> xtk -> oev
        pV1 = ctx.enter_context(tc.tile_pool(name="v1", bufs=8))
        pOsb = ctx.enter_context(tc.tile_pool(name="osb", bufs=8))
        pWqk = ctx.enter_context(tc.tile_pool(name="wqk", bufs=3))
        pStk = ctx.enter_context(tc.tile_pool(name="stk", bufs=5))
        pE = ctx.enter_context(tc.tile_pool(name="e", bufs=4))
        pV2 = ctx.enter_context(tc.tile_pool(name="v2", bufs=16))
        pSm = ctx.enter_context(tc.tile_pool(name="sm", bufs=6))
        pRec = ctx.enter_context(tc.tile_pool(name="rec", bufs=1))  # rec/bc/tmp
        pC = ctx.enter_context(tc.tile_pool(name="const", bufs=1))

        ps_st = ctx.enter_context(tc.tile_pool(name="ps_st", bufs=4, space="PSUM"))
        ps_p12 = ctx.enter_context(tc.tile_pool(name="ps_p12", bufs=2, space="PSUM"))
        ps_sums = ctx.enter_context(tc.tile_pool(name="ps_sums", bufs=2, space="PSUM"))
        ps_proj = ps_st

        cst = pC.tile([128, 320], MDT, tag="cst")
        nc.sync.dma_start(out=cst, in_=d_cst[:, :])
        ones128 = cst[:, 0:128]

        # ---- phase V: V projection (all heads) ----
        xtv = []
        for kk in range(KC):
            t = pA.tile([128, S], MDT, tag="bigA")
            nc.sync.dma_start(out=t, in_=d_xtv[kk])
            xtv.append(t)
        wv = []
        for kk in range(KC):
            t = pB.tile([128, 1024], MDT, tag="bigB")
            nc.sync.dma_start(out=t, in_=d_wv[kk])
            wv.append(t)
        v1 = []
        for t_ in range(TC):
            vt = pV1.tile([128, 1024], MDT, tag="v1")
            for nh in range(2):
                ps = ps_st.tile([128, 512], F32, tag="ps_st")
                for kk in range(KC):
                    nc.tensor.matmul(
                        ps,
                        lhsT=xtv[kk][:, t_ * 128 : (t_ + 1) * 128],
                        rhs=wv[kk][:, nh * 512 : (nh + 1) * 512],
                        start=(kk == 0),
                        stop=(kk == KC - 1),
                    )
                nc.vector.tensor_copy(vt[:, nh * 512 : (nh + 1) * 512], ps)
            v1.append(vt)

        # ---- load XT_q / XT_k (reuse pA / pB slots) ----
        xtq, xtk = [], []
        for kk in range(KC):
            t = pA.tile([128, S], MDT, tag="bigA")
            nc.sync.dma_start(out=t, in_=d_xtq[kk])
            xtq.append(t)
        for kk in range(KC):
            t = pB.tile([128, S], MDT, tag="bigB")
            nc.sync.dma_start(out=t, in_=d_xtk[kk])
            xtk.append(t)

        # ---- attention per head ----
        osb = []
        for h in range(H):
            wqh = pWqk.tile([128, 1024], MDT, tag="wqk")
            nc.sync.dma_start(out=wqh, in_=d_wq[h])
            wkh = pWqk.tile([128, 1024], MDT, tag="wqk")
            nc.sync.dma_start(out=wkh, in_=d_wk[h])

            # Q projection -> qstack [(c,dh)=128, S]
            qstack = pStk.tile([128, S], MDT, tag="stk")
            for nh in range(2):
                ps = ps_proj.tile([128, 512], F32, tag="ps_st")
                for kk in range(KC):
                    nc.tensor.matmul(
                        ps,
                        lhsT=wqh[:, kk * 128 : (kk + 1) * 128],
                        rhs=xtq[kk][:, nh * 512 : (nh + 1) * 512],
                        start=(kk == 0),
                        stop=(kk == KC - 1),
                    )
                nc.vector.tensor_copy(qstack[:, nh * 512 : (nh + 1) * 512], ps)
            # qswap = [qi.T; qr.T] via partition-crossing SBUF->SBUF DMA
            qswap = pStk.tile([128, S], MDT, tag="stk")
            nc.sync.dma_start(out=qswap[0:64, :], in_=qstack[64:128, :])
            nc.sync.dma_start(out=qswap[64:128, :], in_=qstack[0:64, :])

            # K projection -> kstack [kr.T; ki.T], kneg [kr.T; -ki.T]
            kstack = pStk.tile([128, S], MDT, tag="stk")
            kneg = pStk.tile([128, S], MDT, tag="stk")
            for nh in range(2):
                sl = slice(nh * 512, (nh + 1) * 512)
                ps = ps_proj.tile([128, 512], F32, tag="ps_st")
                for kk in range(KC):
                    nc.tensor.matmul(
                        ps,
                        lhsT=wkh[:, kk * 128 : (kk + 1) * 128],
                        rhs=xtk[kk][:, nh * 512 : (nh + 1) * 512],
                        start=(kk == 0),
                        stop=(kk == KC - 1),
                    )
                nc.vector.tensor_copy(kstack[:, sl], ps)
                nc.vector.tensor_copy(kneg[0:64, sl], ps[0:64, :])
                nc.vector.tensor_scalar_mul(kneg[64:128, sl], ps[64:128, :], -1.0)

            # V2_h tiles: [-vi | vr] per tk-chunk
            v2h = []
            for tk in range(TC):
                vt = pV2.tile([128, 128], MDT, tag="v2")
                base = h * 128
                nc.vector.tensor_scalar_mul(
                    vt[:, 0:64], v1[tk][:, base + 64 : base + 128], -1.0
                )
                nc.vector.tensor_copy(vt[:, 64:128], v1[tk][:, base : base + 64])
                v2h.append(vt)

            ot = pOsb.tile([128, S], MDT, tag="osb")
            for nh in range(2):
                nsl = slice(nh * 512, (nh + 1) * 512)
                sums_r = ps_sums.tile([128, 512], F32, tag="ps_sums")
                sums_i = ps_sums.tile([128, 512], F32, tag="ps_sums")
                p1 = ps_p12.tile([128, 512], F32, tag="ps_p12")
                p2 = ps_p12.tile([128, 512], F32, tag="ps_p12")
                for tk in range(TC):
                    ksl = slice(tk * 128, (tk + 1) * 128)
                    for comp in range(2):  # 0: real scores, 1: imag scores
                        lhsT_k = kneg if comp == 0 else kstack
                        rhs_q = qstack if comp == 0 else qswap
                        pdst = p1 if comp == 0 else p2
                        sdst = sums_r if comp == 0 else sums_i
                        vt = v1[tk][:, h * 128 : (h + 1) * 128] if comp == 0 else v2h[tk]
                        st = ps_st.tile([128, 512], F32, tag="ps_st")
                        nc.tensor.matmul(
                            st,
                            lhsT=lhsT_k[:, ksl],
                            rhs=rhs_q[:, nsl],
                            start=True,
                            stop=True,
                        )
                        e = pE.tile([128, 512], MDT, tag="e")
                        nc.scalar.activation(
                            e, st, func=mybir.ActivationFunctionType.Exp
                        )
                        nc.tensor.matmul(
                            sdst,
                            lhsT=ones128,
                            rhs=e,
                            start=(tk == 0),
                            stop=(tk == TC - 1),
                        )
                        nc.tensor.matmul(
                            pdst,
                            lhsT=vt,
                            rhs=e,
                            start=(tk == 0),
                            stop=(tk == TC - 1),
                        )
                rec_r = pSm.tile([128, 512], MDT, tag="sm")
                nc.vector.reciprocal(rec_r, sums_r)
                rec_i = pSm.tile([128, 512], MDT, tag="sm")
                nc.vector.reciprocal(rec_i, sums_i)
                t1 = pSm.tile([128, 512], F32, tag="sm")
                t2 = pSm.tile([128, 512], F32, tag="sm")
                nc.vector.tensor_mul(t1, p1, rec_r)
                nc.vector.tensor_mul(t2, p2, rec_i)
                nc.vector.tensor_add(ot[:, nsl], t1, t2)
            osb.append(ot)

        # ---- output projection ----
        wo = []
        for h in range(H):
            t = pA.tile([128, 1024], MDT, tag="bigA")
            nc.sync.dma_start(out=t, in_=d_wo[h])
            wo.append(t)
        for t_ in range(TC):
            tsl = slice(t_ * 128, (t_ + 1) * 128)
            for nh in range(2):
                nsl = slice(nh * 512, (nh + 1) * 512)
                ps = ps_st.tile([128, 512], F32, tag="ps_st")
                for h in range(H):
                    nc.tensor.matmul(
                        ps,
                        lhsT=osb[h][:, tsl],
                        rhs=wo[h][:, nsl],
                        start=(h == 0),
                        stop=(h == H - 1),
                    )
                oev = pB.tile([128, 512], F32, tag="bigB")
                nc.scalar.copy(oev, ps)
                nc.sync.dma_start(out=d_out[tsl, nsl], in_=oev)

    _split_waits(nc)
    return nc


_NC_CACHE = {}


def kernel(
    queries,
    keys,
    values,
    wq_r,
    wq_i,
    wk_r,
    wk_i,
    wv_r,
    wv_i,
    wo_r,
    wo_i,
    _trace=False,
):
    global LAST_EXEC_NS
    _install_axon_profile_shim()
    _install_tile_drain_patch()
    from concourse.bass_utils import run_bass_kernel_spmd

    scale = 1.0 / np.sqrt(DH)
    WQ = _head_tiles(_build_wqk(np.asarray(wq_r), np.asarray(wq_i), scale))
    WK = _head_tiles(_build_wqk(np.asarray(wk_r), np.asarray(wk_i), 1.0))
    WV = _kchunk_tiles(_build_wqk(np.asarray(wv_r), np.asarray(wv_i), 1.0))
    WO = _kchunk_tiles(_build_wo(np.asarray(wo_r), np.asarray(wo_i)))
    CST = np.zeros((128, 320), np.float32)
    CST[:, 0:128] = 1.0

    queries = np.asarray(queries)
    keys = np.asarray(keys)
    values = np.asarray(values)

    in_maps = []
    for b in range(NCORES):
        in_maps.append(
            {
                "xtq": _xt(queries[b]).reshape(KC, 128, S),
                "xtk": _xt(keys[b]).reshape(KC, 128, S),
                "xtv": _xt(values[b]).reshape(KC, 128, S),
                "wq": WQ,
                "wk": WK,
                "wv": WV,
                "wo": WO,
                "cst": CST,
            }
        )

    if "nc" not in _NC_CACHE:
        _NC_CACHE["nc"] = _build_nc()
    nc = _NC_CACHE["nc"]

    res = run_bass_kernel_spmd(nc, in_maps, list(range(NCORES)), trace=_trace)
    LAST_EXEC_NS = res.exec_time_ns

    out = np.empty((B, S, D, 2), np.float32)
    for b in range(NCORES):
        out[b] = res.results[b]["out"].reshape(S, D, 2)
    return out



# revision 8
# speedup vs baseline: 1.1580x; 1.1580x over previous
"""Complex multi-head attention on 8 Trainium2 cores (Bass/Tile).

Sharding: pure data-parallel over batch (B=8 -> 1 batch per core),
weights replicated. No collectives.

Per-core dataflow (batch b):
  - Host supplies feature-major activations XT = [xr.T; xi.T] [1024, S]
    in bf16 and repacked/sign-folded weights (bf16) so every complex
    linear is one stacked-K real matmul chain.
  - V-projection (all heads) -> V1 token-major [t, (h, vr|vi)] (fp32r).
  - Per head h: Q/K projections -> feature-major stacks [(c,dh)=128, S]
    fp32r; scores computed TRANSPOSED (K-stationary): comp0 lhsT =
    kneg = [kr; -ki], comp1 lhsT = kswap = [ki; kr], rhs = qstack for
    BOTH comps; softmax without max-subtraction (|s| <= ~16);
    row sums via ones-matmuls (partition reduce + broadcast in one);
    reciprocal via the ~5x-faster approx-fast DVE op; normalization
    fused into P1/P2 PSUM evacuation.
  - Software pipelining: head h+1's K-projection block is emitted
    between attn(h, nh=0) and attn(h, nh=1), and its Q-projection
    block after attn(h, nh=1), so the tensor engine never waits on the
    DVE normalize chain or projection evacuations. Head 0's projection
    blocks are interleaved into the V-projection phase. Scores are
    emitted with one-tk lookahead so exp latency is off the PE
    critical path.
  - Output projection accumulates heads as K-chunks -> [t, (o, c)]
    (bf16 osb x bf16 wo), which is exactly the [S, D, 2] DRAM layout.
"""

import sys
import types
import numpy as np
import ml_dtypes

B, S, D, H = 8, 1024, 512, 8
DH = D // H
KC = 8  # k-chunks of 128 over (c,d) = 1024
TC = 8  # token chunks of 128
NCORES = 8

LAST_EXEC_NS = None


# ---------------------------------------------------------------- shims
def _install_axon_profile_shim():
    if "antenv.axon_hooks" in sys.modules:
        return
    try:
        import antenv  # noqa: F401

        mod = types.ModuleType("antenv.axon_hooks")
        state = {"hook": None}
        mod.set_axon_ntff_profile_hook = lambda h: state.__setitem__("hook", h)
        mod.get_axon_ntff_profile_hook = lambda: state["hook"]
        sys.modules["antenv.axon_hooks"] = mod
        from trn_agent_boot.trn_boot import _ntff_profile_via_ctypes

        hook = _ntff_profile_via_ctypes("/opt/axon/libaxon_pjrt.so")
        if hook is not None:
            mod.set_axon_ntff_profile_hook(hook)
    except Exception:
        pass


def _install_tile_drain_patch():
    """This walrus build allows ONE sync wait per instruction; split the
    TileContext exit drain's waits across preceding sync NOPs."""
    import concourse.mybir as mybir
    import concourse.tile as tile
    from concourse.vector_clock import ScopedClock

    if getattr(tile.TileContext, "_drain_patched", False):
        return

    def _patched(self, tick_clock, wait_clock):
        probe = mybir.InstNoOp(name="I-drain-probe")
        probe.engine = mybir.EngineType.SP
        wait_clock.add_sem_waits(probe, ScopedClock({None: tick_clock.global_clock}))
        waits = list(probe.sync_info.on_wait or []) if probe.sync_info else []
        for w in waits:
            nop = self.nc.sync.nop()
            nop.ins.sync_info = mybir.SyncInfo(on_wait=[w], on_update=[])
        self.nc.sync.drain()
        self.nc.all_engine_barrier()
        assert self.sems is not None
        popped = self.nc._tile_sem_poison_stack.pop()
        assert popped is self._sem_poison
        self.nc.clear_and_free_semaphores(list(self.sems.allocated().values()))
        self.nc.all_engine_barrier()

    tile.TileContext._drain_and_barrier = _patched
    tile.TileContext._drain_patched = True


def _split_waits(nc, max_waits=1):
    """Hoist extra sync waits onto preceding same-engine NOPs (walrus here
    rejects >1 sync wait per instruction)."""
    import concourse.mybir as mybir

    def process(blk):
        lst = blk.instructions
        i = 0
        while i < len(lst):
            inst = lst[i]
            if hasattr(inst, "blocks"):
                for b in inst.blocks or []:
                    process(b)
            si = inst.sync_info
            if si is not None and si.on_wait and len(si.on_wait) > max_waits:
                waits = list(si.on_wait)
                keep, extra = waits[-max_waits:], waits[:-max_waits]
                inst.sync_info = mybir.SyncInfo(
                    on_wait=keep, on_update=list(si.on_update or [])
                )
                for j, w in enumerate(extra):
                    nop = mybir.InstNoOp(name=f"{inst.name}-ws{j}")
                    nop.engine = inst.engine
                    nop.sync_info = mybir.SyncInfo(on_wait=[w], on_update=[])
                    lst.insert(i, nop)
                    i += 1
            i += 1

    for f in nc.m.functions:
        for blk in f.blocks:
            process(blk)


# ------------------------------------------------------------ host prep
def _build_wqk(wr, wi, scale):
    """[1024 k=(c,d), 1024 m=(h, c', dh)] for Q/K/V projections."""
    W = np.empty((2 * D, 2 * D), np.float32)
    for h in range(H):
        o = slice(h * DH, (h + 1) * DH)
        c0 = h * 2 * DH
        W[0:D, c0 : c0 + DH] = wr[o].T * scale
        W[D:, c0 : c0 + DH] = -wi[o].T * scale
        W[0:D, c0 + DH : c0 + 2 * DH] = wi[o].T * scale
        W[D:, c0 + DH : c0 + 2 * DH] = wr[o].T * scale
    return W


def _head_tiles(W):
    """[1024,1024] -> [H, 128, 1024]: per-head column block, k-chunk cols."""
    out = np.empty((H, 128, 1024), np.float32)
    for h in range(H):
        blk = W[:, h * 128 : (h + 1) * 128]  # [1024, 128]
        for kk in range(KC):
            out[h, :, kk * 128 : (kk + 1) * 128] = blk[kk * 128 : (kk + 1) * 128]
    return out


def _kchunk_tiles(W):
    """[1024,1024] -> [KC, 128, 1024]: row chunks."""
    return np.ascontiguousarray(W.reshape(KC, 128, 1024))


def _build_wo(wo_r, wo_i):
    """rows (h, c', dh), cols (o, c) interleaved to match [S, D, 2]."""
    W = np.empty((2 * D, 2 * D), np.float32)
    for h in range(H):
        d = slice(h * DH, (h + 1) * DH)
        r0 = h * 2 * DH
        W[r0 : r0 + DH, 0::2] = wo_r[:, d].T
        W[r0 : r0 + DH, 1::2] = wo_i[:, d].T
        W[r0 + DH : r0 + 2 * DH, 0::2] = -wo_i[:, d].T
        W[r0 + DH : r0 + 2 * DH, 1::2] = wo_r[:, d].T
    return W


def _xt(x):  # [S, D, 2] -> [2D, S] feature-major
    out = np.empty((2 * D, S), np.float32)
    out[0:D] = x[:, :, 0].T
    out[D:] = x[:, :, 1].T
    return out


def _bf16(a):
    return np.ascontiguousarray(a).astype(ml_dtypes.bfloat16)


# ------------------------------------------------------------ bass build
def _build_nc():
    import concourse.bass as bass
    import concourse.mybir as mybir
    import concourse.tile as tile
    from contextlib import ExitStack

    MDT = mybir.dt.float32r
    BF = mybir.dt.bfloat16
    F32 = mybir.dt.float32

    nc = bass.Bass()
    d_xtq = nc.dram_tensor("xtq", [KC, 128, S], BF, kind="ExternalInput")
    d_xtk = nc.dram_tensor("xtk", [KC, 128, S], BF, kind="ExternalInput")
    d_xtv = nc.dram_tensor("xtv", [KC, 128, S], BF, kind="ExternalInput")
    d_wq = nc.dram_tensor("wq", [H, 128, 1024], BF, kind="ExternalInput")
    d_wk = nc.dram_tensor("wk", [H, 128, 1024], BF, kind="ExternalInput")
    d_wv = nc.dram_tensor("wv", [KC, 128, 1024], BF, kind="ExternalInput")
    d_wo = nc.dram_tensor("wo", [H, 128, 1024], BF, kind="ExternalInput")
    d_cst = nc.dram_tensor("cst", [128, 128], BF, kind="ExternalInput")
    d_out = nc.dram_tensor("out", [S, 1024], F32, kind="ExternalOutput")

    with tile.TileContext(nc) as tc, ExitStack() as ctx:
        ctx.enter_context(
            nc.allow_low_precision(reason="bf16 projections / fp32r attention")
        )
        pXv = ctx.enter_context(tc.tile_pool(name="xv", bufs=8))
        pXq = ctx.enter_context(tc.tile_pool(name="xq", bufs=8))
        pXk = ctx.enter_context(tc.tile_pool(name="xk", bufs=8))
        pWv = ctx.enter_context(tc.tile_pool(name="wv", bufs=8))
        pWqk = ctx.enter_context(tc.tile_pool(name="wqk", bufs=4))
        pWo = ctx.enter_context(tc.tile_pool(name="wo", bufs=8))
        pV1 = ctx.enter_context(tc.tile_pool(name="v1", bufs=8))
        pV2 = ctx.enter_context(tc.tile_pool(name="v2", bufs=16))
        pOsb = ctx.enter_context(tc.tile_pool(name="osb", bufs=8))
        pStk = ctx.enter_context(tc.tile_pool(name="stk", bufs=8))
        pE = ctx.enter_context(tc.tile_pool(name="e", bufs=14))
        pSm = ctx.enter_context(tc.tile_pool(name="sm", bufs=4))
        pOev = ctx.enter_context(tc.tile_pool(name="oev", bufs=2))
        pC = ctx.enter_context(tc.tile_pool(name="const", bufs=1))

        ps_work = ctx.enter_context(tc.tile_pool(name="ps_work", bufs=4, space="PSUM"))
        ps_p12 = ctx.enter_context(tc.tile_pool(name="ps_p12", bufs=2, space="PSUM"))
        ps_sums = ctx.enter_context(tc.tile_pool(name="ps_sums", bufs=2, space="PSUM"))

        cst = pC.tile([128, 128], BF, tag="cst")
        nc.sync.dma_start(out=cst, in_=d_cst[:, :])
        ones128 = cst

        # ---- input DMA, priority order ----
        xtv = []
        for kk in range(KC):
            t = pXv.tile([128, S], BF, tag="xv")
            nc.sync.dma_start(out=t, in_=d_xtv[kk])
            xtv.append(t)
        wv = []
        for kk in range(KC):
            t = pWv.tile([128, 1024], BF, tag="wv")
            nc.sync.dma_start(out=t, in_=d_wv[kk])
            wv.append(t)
        xtk = []
        for kk in range(KC):
            t = pXk.tile([128, S], BF, tag="xk")
            nc.sync.dma_start(out=t, in_=d_xtk[kk])
            xtk.append(t)
        wk_t = {}
        wq_t = {}
        wk_t[0] = pWqk.tile([128, 1024], BF, tag="wqk", name="wk0")
        nc.sync.dma_start(out=wk_t[0], in_=d_wk[0])
        xtq = []
        for kk in range(KC):
            t = pXq.tile([128, S], BF, tag="xq")
            nc.sync.dma_start(out=t, in_=d_xtq[kk])
            xtq.append(t)
        wq_t[0] = pWqk.tile([128, 1024], BF, tag="wqk", name="wq0")
        nc.sync.dma_start(out=wq_t[0], in_=d_wq[0])

        # per-head state
        qstack = {}
        kneg = {}
        kswap = {}
        v2h = {}
        v1 = []
        osb = []
        wo_t = []

        def proj_k_gen(h):
            """K projection for head h -> kneg [kr;-ki], kswap [ki;kr].
            Yields every 4 matmuls so callers can interleave PE work."""
            kst = pStk.tile([128, S], MDT, tag="stk", name=f"kst{h}")
            kng = pStk.tile([128, S], MDT, tag="stk", name=f"kng{h}")
            ksw = pStk.tile([128, S], MDT, tag="stk", name=f"ksw{h}")
            kneg[h] = kng
            kswap[h] = ksw
            for nh in range(2):
                sl = slice(nh * 512, (nh + 1) * 512)
                ps = ps_work.tile([128, 512], F32, tag="ps_work", name=f"psk{h}{nh}")
                for kk in range(KC):
                    nc.tensor.matmul(
                        ps,
                        lhsT=wk_t[h][:, kk * 128 : (kk + 1) * 128],
                        rhs=xtk[kk][:, sl],
                        start=(kk == 0),
                        stop=(kk == KC - 1),
                    )
                    if kk % 4 == 3:
                        yield
                nc.vector.tensor_copy(kst[:, sl], ps)
                nc.vector.tensor_copy(kng[0:64, sl], ps[0:64, :])
                nc.vector.tensor_scalar_mul(kng[64:128, sl], ps[64:128, :], -1.0)
            nc.sync.dma_start(out=ksw[0:64, :], in_=kst[64:128, :])
            nc.sync.dma_start(out=ksw[64:128, :], in_=kst[0:64, :])

        def proj_q_gen(h):
            qst = pStk.tile([128, S], MDT, tag="stk", name=f"qst{h}")
            qstack[h] = qst
            for nh in range(2):
                sl = slice(nh * 512, (nh + 1) * 512)
                ps = ps_work.tile([128, 512], F32, tag="ps_work", name=f"psq{h}{nh}")
                for kk in range(KC):
                    nc.tensor.matmul(
                        ps,
                        lhsT=wq_t[h][:, kk * 128 : (kk + 1) * 128],
                        rhs=xtq[kk][:, sl],
                        start=(kk == 0),
                        stop=(kk == KC - 1),
                    )
                    if kk % 4 == 3:
                        yield
                nc.vector.tensor_copy(qst[:, sl], ps)

        def emit_v2(h):
            lst = []
            base = h * 128
            for tk in range(TC):
                vt = pV2.tile([128, 128], BF, tag="v2")
                nc.vector.tensor_scalar_mul(
                    vt[:, 0:64], v1[tk][:, base + 64 : base + 128], -1.0
                )
                nc.vector.tensor_copy(vt[:, 64:128], v1[tk][:, base : base + 64])
                lst.append(vt)
            v2h[h] = lst

        # ---- V projection with head-0 proj blocks interleaved ----
        for t_ in range(TC):
            vt = pV1.tile([128, 1024], BF, tag="v1")
            for nh in range(2):
                ps = ps_work.tile([128, 512], F32, tag="ps_work")
                for kk in range(KC):
                    nc.tensor.matmul(
                        ps,
                        lhsT=xtv[kk][:, t_ * 128 : (t_ + 1) * 128],
                        rhs=wv[kk][:, nh * 512 : (nh + 1) * 512],
                        start=(kk == 0),
                        stop=(kk == KC - 1),
                    )
                nc.vector.tensor_copy(vt[:, nh * 512 : (nh + 1) * 512], ps)
            v1.append(vt)
            if t_ == 3:
                pk0 = proj_k_gen(0)
                next(pk0, None)
                next(pk0, None)
            elif t_ == 4:
                for _ in pk0:
                    pass
            elif t_ == 5:
                pq0 = proj_q_gen(0)
                next(pq0, None)
                next(pq0, None)
            elif t_ == 6:
                for _ in pq0:
                    pass
        emit_v2(0)

        # ---- attention per head, software pipelined ----
        def emit_attn(h, nh, ot, gen=None):
            nsl = slice(nh * 512, (nh + 1) * 512)
            sums_r = ps_sums.tile([128, 512], F32, tag="ps_sums")
            sums_i = ps_sums.tile([128, 512], F32, tag="ps_sums")
            p1 = ps_p12.tile([128, 512], F32, tag="ps_p12")
            p2 = ps_p12.tile([128, 512], F32, tag="ps_p12")
            es = {}

            def emit_st(tk):
                ksl = slice(tk * 128, (tk + 1) * 128)
                for comp, lhsT_k in ((0, kneg[h]), (1, kswap[h])):
                    st = ps_work.tile([128, 512], F32, tag="ps_work")
                    nc.tensor.matmul(
                        st, lhsT=lhsT_k[:, ksl], rhs=qstack[h][:, nsl],
                        start=True, stop=True,
                    )
                    e = pE.tile([128, 512], BF, tag="e")
                    nc.scalar.activation(e, st, func=mybir.ActivationFunctionType.Exp)
                    es[(tk, comp)] = e

            def emit_sums_av(tk):
                for comp in range(2):
                    e = es.pop((tk, comp))
                    sdst = sums_r if comp == 0 else sums_i
                    pdst = p1 if comp == 0 else p2
                    vt = (
                        v1[tk][:, h * 128 : (h + 1) * 128]
                        if comp == 0
                        else v2h[h][tk]
                    )
                    nc.tensor.matmul(
                        sdst, lhsT=ones128, rhs=e,
                        start=(tk == 0), stop=(tk == TC - 1),
                    )
                    nc.tensor.matmul(
                        pdst, lhsT=vt, rhs=e,
                        start=(tk == 0), stop=(tk == TC - 1),
                    )

            LOOK = 6
            emit_st(0)
            emit_st(1)
            for i in range(2, LOOK):
                if gen is not None:
                    next(gen, None)
                emit_st(i)
            if gen is not None:
                for _ in gen:
                    pass
            for tk in range(TC):
                if tk + LOOK < TC:
                    emit_st(tk + LOOK)
                emit_sums_av(tk)

            rec_r = pSm.tile([128, 512], F32, tag="sm")
            nc.vector.reciprocal(rec_r, sums_r)
            t1 = pSm.tile([128, 512], F32, tag="sm")
            nc.vector.tensor_mul(t1, p1, rec_r)
            rec_i = pSm.tile([128, 512], F32, tag="sm")
            nc.vector.reciprocal(rec_i, sums_i)
            t2 = pSm.tile([128, 512], F32, tag="sm")
            nc.vector.tensor_mul(t2, p2, rec_i)
            nc.vector.tensor_add(ot[:, nsl], t1, t2)

        for h in range(H):
            ot = pOsb.tile([128, S], BF, tag="osb")
            if h + 1 < H:
                wk_t[h + 1] = pWqk.tile([128, 1024], BF, tag="wqk", name=f"wk{h+1}")
                nc.sync.dma_start(out=wk_t[h + 1], in_=d_wk[h + 1])
                wq_t[h + 1] = pWqk.tile([128, 1024], BF, tag="wqk", name=f"wq{h+1}")
                nc.sync.dma_start(out=wq_t[h + 1], in_=d_wq[h + 1])
                emit_v2(h + 1)
            if h == 5:
                # wo DMA: late, overlaps heads 6-7
                for hh in range(H):
                    t = pWo.tile([128, 1024], BF, tag="wo")
                    nc.sync.dma_start(out=t, in_=d_wo[hh])
                    wo_t.append(t)
            emit_attn(h, 0, ot, proj_k_gen(h + 1) if h + 1 < H else None)
            emit_attn(h, 1, ot, proj_q_gen(h + 1) if h + 1 < H else None)
            osb.append(ot)

        # ---- output projection ----
        for t_ in range(TC):
            tsl = slice(t_ * 128, (t_ + 1) * 128)
            for nh in range(2):
                nsl = slice(nh * 512, (nh + 1) * 512)
                ps = ps_work.tile([128, 512], F32, tag="ps_work")
                for h in range(H):
                    nc.tensor.matmul(
                        ps,
                        lhsT=osb[h][:, tsl],
                        rhs=wo_t[h][:, nsl],
                        start=(h == 0),
                        stop=(h == H - 1),
                    )
                oev = pOev.tile([128, 512], F32, tag="oev")
                nc.scalar.copy(oev, ps)
                nc.sync.dma_start(out=d_out[tsl, nsl], in_=oev)

    _split_waits(nc)
    return nc


_NC_CACHE = {}


def kernel(
    queries,
    keys,
    values,
    wq_r,
    wq_i,
    wk_r,
    wk_i,
    wv_r,
    wv_i,
    wo_r,
    wo_i,
    _trace=False,
):
    global LAST_EXEC_NS
    _install_axon_profile_shim()
    _install_tile_drain_patch()
    from concourse.bass_utils import run_bass_kernel_spmd

    scale = 1.0 / np.sqrt(DH)
    WQ = _bf16(_head_tiles(_build_wqk(np.asarray(wq_r), np.asarray(wq_i), scale)))
    WK = _bf16(_head_tiles(_build_wqk(np.asarray(wk_r), np.asarray(wk_i), 1.0)))
    WV = _bf16(_kchunk_tiles(_build_wqk(np.asarray(wv_r), np.asarray(wv_i), 1.0)))
    WO = _bf16(_kchunk_tiles(_build_wo(np.asarray(wo_r), np.asarray(wo_i))))
    CST = np.ones((128, 128), ml_dtypes.bfloat16)

    queries = np.asarray(queries)
    keys = np.asarray(keys)
    values = np.asarray(values)

    in_maps = []
    for b in range(NCORES):
        in_maps.append(
            {
                "xtq": _bf16(_xt(queries[b]).reshape(KC, 128, S)),
                "xtk": _bf16(_xt(keys[b]).reshape(KC, 128, S)),
                "xtv": _bf16(_xt(values[b]).reshape(KC, 128, S)),
                "wq": WQ,
                "wk": WK,
                "wv": WV,
                "wo": WO,
                "cst": CST,
            }
        )

    if "nc" not in _NC_CACHE:
        _NC_CACHE["nc"] = _build_nc()
    nc = _NC_CACHE["nc"]

    res = run_bass_kernel_spmd(nc, in_maps, list(range(NCORES)), trace=_trace)
    LAST_EXEC_NS = res.exec_time_ns

    out = np.empty((B, S, D, 2), np.float32)
    for b in range(NCORES):
        out[b] = res.results[b]["out"].reshape(S, D, 2)
    return out


# revision 10
# speedup vs baseline: 1.2864x; 1.1109x over previous
"""Complex multi-head attention on 8 Trainium2 cores (Bass/Tile).

Sharding: pure data-parallel over batch (B=8 -> 1 batch per core),
weights replicated. No collectives.

Per-core dataflow (batch b):
  - Host supplies feature-major activations XT = [xr.T; xi.T] [1024, S]
    in bf16 and repacked/sign-folded weights (bf16) so every complex
    linear is one stacked-K real matmul chain.
  - V-projection (all heads) -> V1 token-major [t, (h, vr|vi)] (fp32r).
  - Per head h: Q/K projections -> feature-major stacks [(c,dh)=128, S]
    fp32r; scores computed TRANSPOSED (K-stationary): comp0 lhsT =
    kneg = [kr; -ki], comp1 lhsT = kswap = [ki; kr], rhs = qstack for
    BOTH comps; softmax without max-subtraction (|s| <= ~16);
    row sums via ones-matmuls (partition reduce + broadcast in one);
    reciprocal via the ~5x-faster approx-fast DVE op; normalization
    fused into P1/P2 PSUM evacuation.
  - Software pipelining: head h+1's K-projection block is emitted
    between attn(h, nh=0) and attn(h, nh=1), and its Q-projection
    block after attn(h, nh=1), so the tensor engine never waits on the
    DVE normalize chain or projection evacuations. Head 0's projection
    blocks are interleaved into the V-projection phase. Scores are
    emitted with one-tk lookahead so exp latency is off the PE
    critical path.
  - Output projection accumulates heads as K-chunks -> [t, (o, c)]
    (bf16 osb x bf16 wo), which is exactly the [S, D, 2] DRAM layout.
"""

import sys
import types
import numpy as np
import ml_dtypes

B, S, D, H = 8, 1024, 512, 8
DH = D // H
KC = 8  # k-chunks of 128 over (c,d) = 1024
TC = 8  # token chunks of 128
NCORES = 8

LAST_EXEC_NS = None


# ---------------------------------------------------------------- shims
def _install_axon_profile_shim():
    if "antenv.axon_hooks" in sys.modules:
        return
    try:
        import antenv  # noqa: F401

        mod = types.ModuleType("antenv.axon_hooks")
        state = {"hook": None}
        mod.set_axon_ntff_profile_hook = lambda h: state.__setitem__("hook", h)
        mod.get_axon_ntff_profile_hook = lambda: state["hook"]
        sys.modules["antenv.axon_hooks"] = mod
        from trn_agent_boot.trn_boot import _ntff_profile_via_ctypes

        hook = _ntff_profile_via_ctypes("/opt/axon/libaxon_pjrt.so")
        if hook is not None:
            mod.set_axon_ntff_profile_hook(hook)
    except Exception:
        pass


def _install_tile_drain_patch():
    """This walrus build allows ONE sync wait per instruction; split the
    TileContext exit drain's waits across preceding sync NOPs."""
    import concourse.mybir as mybir
    import concourse.tile as tile
    from concourse.vector_clock import ScopedClock

    if getattr(tile.TileContext, "_drain_patched", False):
        return

    def _patched(self, tick_clock, wait_clock):
        probe = mybir.InstNoOp(name="I-drain-probe")
        probe.engine = mybir.EngineType.SP
        wait_clock.add_sem_waits(probe, ScopedClock({None: tick_clock.global_clock}))
        waits = list(probe.sync_info.on_wait or []) if probe.sync_info else []
        for w in waits:
            nop = self.nc.sync.nop()
            nop.ins.sync_info = mybir.SyncInfo(on_wait=[w], on_update=[])
        self.nc.sync.drain()
        self.nc.all_engine_barrier()
        assert self.sems is not None
        popped = self.nc._tile_sem_poison_stack.pop()
        assert popped is self._sem_poison
        self.nc.clear_and_free_semaphores(list(self.sems.allocated().values()))
        self.nc.all_engine_barrier()

    tile.TileContext._drain_and_barrier = _patched
    tile.TileContext._drain_patched = True


def _split_waits(nc, max_waits=1):
    """Hoist extra sync waits onto preceding same-engine NOPs (walrus here
    rejects >1 sync wait per instruction)."""
    import concourse.mybir as mybir

    def process(blk):
        lst = blk.instructions
        i = 0
        while i < len(lst):
            inst = lst[i]
            if hasattr(inst, "blocks"):
                for b in inst.blocks or []:
                    process(b)
            si = inst.sync_info
            if si is not None and si.on_wait and len(si.on_wait) > max_waits:
                waits = list(si.on_wait)
                keep, extra = waits[-max_waits:], waits[:-max_waits]
                inst.sync_info = mybir.SyncInfo(
                    on_wait=keep, on_update=list(si.on_update or [])
                )
                for j, w in enumerate(extra):
                    nop = mybir.InstNoOp(name=f"{inst.name}-ws{j}")
                    nop.engine = inst.engine
                    nop.sync_info = mybir.SyncInfo(on_wait=[w], on_update=[])
                    lst.insert(i, nop)
                    i += 1
            i += 1

    for f in nc.m.functions:
        for blk in f.blocks:
            process(blk)


# ------------------------------------------------------------ host prep
def _build_wqk(wr, wi, scale):
    """[1024 k=(c,d), 1024 m=(h, c', dh)] for Q/K/V projections."""
    W = np.empty((2 * D, 2 * D), np.float32)
    for h in range(H):
        o = slice(h * DH, (h + 1) * DH)
        c0 = h * 2 * DH
        W[0:D, c0 : c0 + DH] = wr[o].T * scale
        W[D:, c0 : c0 + DH] = -wi[o].T * scale
        W[0:D, c0 + DH : c0 + 2 * DH] = wi[o].T * scale
        W[D:, c0 + DH : c0 + 2 * DH] = wr[o].T * scale
    return W


def _head_tiles(W):
    """[1024,1024] -> [H, 128, 1024]: per-head column block, k-chunk cols."""
    out = np.empty((H, 128, 1024), np.float32)
    for h in range(H):
        blk = W[:, h * 128 : (h + 1) * 128]  # [1024, 128]
        for kk in range(KC):
            out[h, :, kk * 128 : (kk + 1) * 128] = blk[kk * 128 : (kk + 1) * 128]
    return out


def _kchunk_tiles(W):
    """[1024,1024] -> [KC, 128, 1024]: row chunks."""
    return np.ascontiguousarray(W.reshape(KC, 128, 1024))


def _build_wo(wo_r, wo_i):
    """rows (h, c', dh), cols (o, c) interleaved to match [S, D, 2]."""
    W = np.empty((2 * D, 2 * D), np.float32)
    for h in range(H):
        d = slice(h * DH, (h + 1) * DH)
        r0 = h * 2 * DH
        W[r0 : r0 + DH, 0::2] = wo_r[:, d].T
        W[r0 : r0 + DH, 1::2] = wo_i[:, d].T
        W[r0 + DH : r0 + 2 * DH, 0::2] = -wo_i[:, d].T
        W[r0 + DH : r0 + 2 * DH, 1::2] = wo_r[:, d].T
    return W


def _xt(x):  # [S, D, 2] -> [2D, S] feature-major
    out = np.empty((2 * D, S), np.float32)
    out[0:D] = x[:, :, 0].T
    out[D:] = x[:, :, 1].T
    return out


def _bf16(a):
    return np.ascontiguousarray(a).astype(ml_dtypes.bfloat16)


# ------------------------------------------------------------ bass build
def _build_nc():
    import concourse.bass as bass
    import concourse.mybir as mybir
    import concourse.tile as tile
    from contextlib import ExitStack

    MDT = mybir.dt.float32r
    BF = mybir.dt.bfloat16
    F32 = mybir.dt.float32

    nc = bass.Bass()
    d_xtq = nc.dram_tensor("xtq", [KC, 128, S], BF, kind="ExternalInput")
    d_xtk = nc.dram_tensor("xtk", [KC, 128, S], BF, kind="ExternalInput")
    d_xtv = nc.dram_tensor("xtv", [KC, 128, S], BF, kind="ExternalInput")
    d_wq = nc.dram_tensor("wq", [H, 128, 1024], BF, kind="ExternalInput")
    d_wk = nc.dram_tensor("wk", [H, 128, 1024], BF, kind="ExternalInput")
    d_wv = nc.dram_tensor("wv", [KC, 128, 1024], BF, kind="ExternalInput")
    d_wo = nc.dram_tensor("wo", [H, 128, 1024], BF, kind="ExternalInput")
    d_cst = nc.dram_tensor("cst", [128, 128], BF, kind="ExternalInput")
    d_out = nc.dram_tensor("out", [S, 1024], F32, kind="ExternalOutput")

    with tile.TileContext(nc) as tc, ExitStack() as ctx:
        ctx.enter_context(
            nc.allow_low_precision(reason="bf16 projections / fp32r attention")
        )
        pXv = ctx.enter_context(tc.tile_pool(name="xv", bufs=8))
        pXq = ctx.enter_context(tc.tile_pool(name="xq", bufs=8))
        pXk = ctx.enter_context(tc.tile_pool(name="xk", bufs=8))
        pWv = ctx.enter_context(tc.tile_pool(name="wv", bufs=8))
        pWqk = ctx.enter_context(tc.tile_pool(name="wqk", bufs=4))
        pWo = ctx.enter_context(tc.tile_pool(name="wo", bufs=8))
        pV1 = ctx.enter_context(tc.tile_pool(name="v1", bufs=8))
        pV2 = ctx.enter_context(tc.tile_pool(name="v2", bufs=16))
        pOsb = ctx.enter_context(tc.tile_pool(name="osb", bufs=8))
        pStk = ctx.enter_context(tc.tile_pool(name="stk", bufs=8))
        pE = ctx.enter_context(tc.tile_pool(name="e", bufs=18))
        pSm = ctx.enter_context(tc.tile_pool(name="sm", bufs=8))
        pOev = ctx.enter_context(tc.tile_pool(name="oev", bufs=2))
        pC = ctx.enter_context(tc.tile_pool(name="const", bufs=1))

        ps_work = ctx.enter_context(tc.tile_pool(name="ps_work", bufs=4, space="PSUM"))
        ps_p12 = ctx.enter_context(tc.tile_pool(name="ps_p12", bufs=2, space="PSUM"))
        ps_sums = ctx.enter_context(tc.tile_pool(name="ps_sums", bufs=2, space="PSUM"))

        cst = pC.tile([128, 128], BF, tag="cst")
        nc.sync.dma_start(out=cst, in_=d_cst[:, :])
        ones128 = cst

        # ---- input DMA, priority order ----
        xtv = []
        for kk in range(KC):
            t = pXv.tile([128, S], BF, tag="xv")
            nc.sync.dma_start(out=t, in_=d_xtv[kk])
            xtv.append(t)
        wv = []
        for kk in range(KC):
            t = pWv.tile([128, 1024], BF, tag="wv")
            nc.sync.dma_start(out=t, in_=d_wv[kk])
            wv.append(t)
        xtk = []
        for kk in range(KC):
            t = pXk.tile([128, S], BF, tag="xk")
            nc.sync.dma_start(out=t, in_=d_xtk[kk])
            xtk.append(t)
        wk_t = {}
        wq_t = {}
        wk_t[0] = pWqk.tile([128, 1024], BF, tag="wqk", name="wk0")
        nc.sync.dma_start(out=wk_t[0], in_=d_wk[0])
        xtq = []
        for kk in range(KC):
            t = pXq.tile([128, S], BF, tag="xq")
            nc.sync.dma_start(out=t, in_=d_xtq[kk])
            xtq.append(t)
        wq_t[0] = pWqk.tile([128, 1024], BF, tag="wqk", name="wq0")
        nc.sync.dma_start(out=wq_t[0], in_=d_wq[0])

        # per-head state
        qstack = {}
        kneg = {}
        kswap = {}
        v2h = {}
        v1 = []
        osb = []
        wo_t = []

        def proj_k_gen(h):
            """K projection for head h -> kneg [kr;-ki], kswap [ki;kr].
            Yields every 4 matmuls so callers can interleave PE work."""
            kst = pStk.tile([128, S], MDT, tag="stk", name=f"kst{h}")
            kng = pStk.tile([128, S], MDT, tag="stk", name=f"kng{h}")
            ksw = pStk.tile([128, S], MDT, tag="stk", name=f"ksw{h}")
            kneg[h] = kng
            kswap[h] = ksw
            for nh in range(2):
                sl = slice(nh * 512, (nh + 1) * 512)
                ps = ps_work.tile([128, 512], F32, tag="ps_work", name=f"psk{h}{nh}")
                for kk in range(KC):
                    nc.tensor.matmul(
                        ps,
                        lhsT=wk_t[h][:, kk * 128 : (kk + 1) * 128],
                        rhs=xtk[kk][:, sl],
                        start=(kk == 0),
                        stop=(kk == KC - 1),
                    )
                    if kk % 4 == 3:
                        yield
                nc.vector.tensor_copy(kst[:, sl], ps)
                nc.vector.tensor_copy(kng[0:64, sl], ps[0:64, :])
                nc.vector.tensor_scalar_mul(kng[64:128, sl], ps[64:128, :], -1.0)
            nc.sync.dma_start(out=ksw[0:64, :], in_=kst[64:128, :])
            nc.sync.dma_start(out=ksw[64:128, :], in_=kst[0:64, :])

        def proj_q_gen(h):
            qst = pStk.tile([128, S], MDT, tag="stk", name=f"qst{h}")
            qstack[h] = qst
            for nh in range(2):
                sl = slice(nh * 512, (nh + 1) * 512)
                ps = ps_work.tile([128, 512], F32, tag="ps_work", name=f"psq{h}{nh}")
                for kk in range(KC):
                    nc.tensor.matmul(
                        ps,
                        lhsT=wq_t[h][:, kk * 128 : (kk + 1) * 128],
                        rhs=xtq[kk][:, sl],
                        start=(kk == 0),
                        stop=(kk == KC - 1),
                    )
                    if kk % 4 == 3:
                        yield
                nc.vector.tensor_copy(qst[:, sl], ps)

        def emit_v2(h):
            lst = []
            base = h * 128
            for tk in range(TC):
                vt = pV2.tile([128, 128], BF, tag="v2")
                nc.vector.tensor_scalar_mul(
                    vt[:, 0:64], v1[tk][:, base + 64 : base + 128], -1.0
                )
                nc.vector.tensor_copy(vt[:, 64:128], v1[tk][:, base : base + 64])
                lst.append(vt)
            v2h[h] = lst

        # ---- V projection with head-0 proj blocks interleaved ----
        for t_ in range(TC):
            vt = pV1.tile([128, 1024], BF, tag="v1")
            for nh in range(2):
                ps = ps_work.tile([128, 512], F32, tag="ps_work")
                for kk in range(KC):
                    nc.tensor.matmul(
                        ps,
                        lhsT=xtv[kk][:, t_ * 128 : (t_ + 1) * 128],
                        rhs=wv[kk][:, nh * 512 : (nh + 1) * 512],
                        start=(kk == 0),
                        stop=(kk == KC - 1),
                    )
                nc.vector.tensor_copy(vt[:, nh * 512 : (nh + 1) * 512], ps)
            v1.append(vt)
            if t_ == 3:
                pk0 = proj_k_gen(0)
                next(pk0, None)
                next(pk0, None)
            elif t_ == 4:
                for _ in pk0:
                    pass
            elif t_ == 5:
                pq0 = proj_q_gen(0)
                next(pq0, None)
                next(pq0, None)
            elif t_ == 6:
                for _ in pq0:
                    pass
        emit_v2(0)

        # ---- attention per head, software pipelined ----
        def emit_attn(h, nh, ot, gen=None):
            nsl = slice(nh * 512, (nh + 1) * 512)
            sums_r = ps_sums.tile([128, 512], F32, tag="ps_sums")
            sums_i = ps_sums.tile([128, 512], F32, tag="ps_sums")
            p1 = ps_p12.tile([128, 512], F32, tag="ps_p12")
            p2 = ps_p12.tile([128, 512], F32, tag="ps_p12")
            es = {}

            def emit_st(tk):
                ksl = slice(tk * 128, (tk + 1) * 128)
                for comp, lhsT_k in ((0, kneg[h]), (1, kswap[h])):
                    st = ps_work.tile([128, 512], F32, tag="ps_work")
                    nc.tensor.matmul(
                        st, lhsT=lhsT_k[:, ksl], rhs=qstack[h][:, nsl],
                        start=True, stop=True,
                    )
                    e = pE.tile([128, 512], BF, tag="e")
                    nc.scalar.activation(e, st, func=mybir.ActivationFunctionType.Exp)
                    es[(tk, comp)] = e

            def emit_sums_av(tk):
                for comp in range(2):
                    e = es.pop((tk, comp))
                    sdst = sums_r if comp == 0 else sums_i
                    pdst = p1 if comp == 0 else p2
                    vt = (
                        v1[tk][:, h * 128 : (h + 1) * 128]
                        if comp == 0
                        else v2h[h][tk]
                    )
                    nc.tensor.matmul(
                        sdst, lhsT=ones128, rhs=e,
                        start=(tk == 0), stop=(tk == TC - 1),
                    )
                    nc.tensor.matmul(
                        pdst, lhsT=vt, rhs=e,
                        start=(tk == 0), stop=(tk == TC - 1),
                    )

            emit_st(0)
            emit_st(1)
            for i in range(2, 8):
                if gen is not None:
                    next(gen, None)
                emit_st(i)
            if gen is not None:
                for _ in gen:
                    pass
            for tk in range(TC):
                emit_sums_av(tk)

            # scalar evacuates P1/P2 (frees PSUM banks immediately);
            # recips on DVE; muls/add on the idle GpSimd engine (SBUF only)
            s1 = pSm.tile([128, 512], F32, tag="sm")
            nc.scalar.copy(s1, p1)
            s2 = pSm.tile([128, 512], F32, tag="sm")
            nc.scalar.copy(s2, p2)
            rec_r = pSm.tile([128, 512], F32, tag="sm")
            nc.vector.reciprocal(rec_r, sums_r)
            t1 = pSm.tile([128, 512], F32, tag="sm")
            nc.gpsimd.tensor_mul(t1, s1, rec_r)
            rec_i = pSm.tile([128, 512], F32, tag="sm")
            nc.vector.reciprocal(rec_i, sums_i)
            t2 = pSm.tile([128, 512], F32, tag="sm")
            nc.gpsimd.tensor_mul(t2, s2, rec_i)
            nc.gpsimd.tensor_add(ot[:, nsl], t1, t2)

        for h in range(H):
            ot = pOsb.tile([128, S], BF, tag="osb")
            if h + 1 < H:
                wk_t[h + 1] = pWqk.tile([128, 1024], BF, tag="wqk", name=f"wk{h+1}")
                nc.sync.dma_start(out=wk_t[h + 1], in_=d_wk[h + 1])
                wq_t[h + 1] = pWqk.tile([128, 1024], BF, tag="wqk", name=f"wq{h+1}")
                nc.sync.dma_start(out=wq_t[h + 1], in_=d_wq[h + 1])
                emit_v2(h + 1)
            if h == 5:
                # wo DMA: late, overlaps heads 6-7
                for hh in range(H):
                    t = pWo.tile([128, 1024], BF, tag="wo")
                    nc.sync.dma_start(out=t, in_=d_wo[hh])
                    wo_t.append(t)
            emit_attn(h, 0, ot, proj_k_gen(h + 1) if h + 1 < H else None)
            emit_attn(h, 1, ot, proj_q_gen(h + 1) if h + 1 < H else None)
            osb.append(ot)

        # ---- output projection ----
        for t_ in range(TC):
            tsl = slice(t_ * 128, (t_ + 1) * 128)
            for nh in range(2):
                nsl = slice(nh * 512, (nh + 1) * 512)
                ps = ps_work.tile([128, 512], F32, tag="ps_work")
                for h in range(H):
                    nc.tensor.matmul(
                        ps,
                        lhsT=osb[h][:, tsl],
                        rhs=wo_t[h][:, nsl],
                        start=(h == 0),
                        stop=(h == H - 1),
                    )
                oev = pOev.tile([128, 512], F32, tag="oev")
                nc.scalar.copy(oev, ps)
                nc.sync.dma_start(out=d_out[tsl, nsl], in_=oev)

    _split_waits(nc)
    return nc


_NC_CACHE = {}


def kernel(
    queries,
    keys,
    values,
    wq_r,
    wq_i,
    wk_r,
    wk_i,
    wv_r,
    wv_i,
    wo_r,
    wo_i,
    _trace=False,
):
    global LAST_EXEC_NS
    _install_axon_profile_shim()
    _install_tile_drain_patch()
    from concourse.bass_utils import run_bass_kernel_spmd

    scale = 1.0 / np.sqrt(DH)
    WQ = _bf16(_head_tiles(_build_wqk(np.asarray(wq_r), np.asarray(wq_i), scale)))
    WK = _bf16(_head_tiles(_build_wqk(np.asarray(wk_r), np.asarray(wk_i), 1.0)))
    WV = _bf16(_kchunk_tiles(_build_wqk(np.asarray(wv_r), np.asarray(wv_i), 1.0)))
    WO = _bf16(_kchunk_tiles(_build_wo(np.asarray(wo_r), np.asarray(wo_i))))
    CST = np.ones((128, 128), ml_dtypes.bfloat16)

    queries = np.asarray(queries)
    keys = np.asarray(keys)
    values = np.asarray(values)

    in_maps = []
    for b in range(NCORES):
        in_maps.append(
            {
                "xtq": _bf16(_xt(queries[b]).reshape(KC, 128, S)),
                "xtk": _bf16(_xt(keys[b]).reshape(KC, 128, S)),
                "xtv": _bf16(_xt(values[b]).reshape(KC, 128, S)),
                "wq": WQ,
                "wk": WK,
                "wv": WV,
                "wo": WO,
                "cst": CST,
            }
        )

    if "nc" not in _NC_CACHE:
        _NC_CACHE["nc"] = _build_nc()
    nc = _NC_CACHE["nc"]

    res = run_bass_kernel_spmd(nc, in_maps, list(range(NCORES)), trace=_trace)
    LAST_EXEC_NS = res.exec_time_ns

    out = np.empty((B, S, D, 2), np.float32)
    for b in range(NCORES):
        out[b] = res.results[b]["out"].reshape(S, D, 2)
    return out
